# revision 17
# baseline (speedup 1.0000x reference)
"""LSTM encoder (T=512, B=256, H=256, V=32000) on 8 trn2 NeuronCores.

Strategy
--------
Data-parallel over batch: B=256 -> 32 per core; weights/table replicated.

Per core the recurrence runs in a transposed "gatesT" layout: gates live as
[4H on partitions (8 chunks of 128), batch in the free dim]. Weight chunks
are the stationary matmul operand (fp16), h streams as the moving operand.

All 32 lanes run as ONE chain (a 2-chain batch split doubles the number of
fixed-latency-dominated ACT/DVE ops per step and saturates the scalar
engine; the cycle time is set by the serial chain either way). The
per-step serial chain is

  W_hh burst -> sigma(f,i) -> f*c -> i*tanh(g) -> add -> tanh(c)
  -> h = o*tanh(c) -> next burst

with measured-trace-driven choices:

- gate chunk order is [f, i, g, o] host-side; the burst emits the f,i
  matmuls FIRST (sigma(f,i) is the chain head), then g, then o. The g-gate
  gets a direct Tanh activation that runs on the ACT engine WHILE the DVE
  computes f*c, so only ~190ns of the tanh is exposed on the chain.
- sigma(o) only gates the final h-mul and runs in ACT idle time.
- each window's PSUM is split into per-gate-group tiles (f,i | g | o)
  because Tile tracks PSUM hazards per-tile: with one tile the gate
  activations would wait on the whole burst (and the burst on prior
  activations), serializing the step at ~3.4us.
- activations are emitted interleaved into the burst segments so their
  PE-semaphore wait thresholds stay minimal (Tile computes thresholds
  from program order).
- sigma outputs, tanh(g) and tanh(c) are fp16 so i*tanh(g) and
  h = o*tanh(c) run in the DVE 2x half-precision mode. c stays fp32
  (512-step accumulation).

The input projections W_ih @ emb_t for a window of 8 future steps are
precomputed into the window's PSUM banks (one [128, M, S, BL] = 4-bank tile
per window; two windows in flight = all 8 banks). The first write to each
bank is a full-bank N=512 bias matmul with start=True; x-projection
matmuls (N=256, one per weight chunk) and the per-step W_hh bursts
accumulate on top (start=False). They are pinned behind each step's
recurrent burst so they fill the PE-idle tail of the step.

Embeddings are fetched with dma_gather(transpose=True): gathers fp16 table
rows and deposits them H-on-partitions, the exact rhs layout the
X-projection matmuls need.

Numerics: fp16 table/weights/h (matmul operands), fp32 PSUM and fp32
elementwise state c. Expected absmax/scale error ~5e-4 vs fp32 reference.
"""

import numpy as np

T, B, H, V = 512, 256, 256, 32000
N_CORES = 8
BL = B // N_CORES          # 32 batch per core
S = 8                      # steps per PSUM window
G4 = 4 * H                 # 1024
M = G4 // 128              # 8 gate chunks
K = H // 128               # 2 contraction chunks

# gate chunk order f, i, g, o (PyTorch native is i, f, g, o)
_PERM = np.concatenate([
    np.arange(H, 2 * H),       # f
    np.arange(0, H),           # i
    np.arange(2 * H, 3 * H),   # g
    np.arange(3 * H, 4 * H),   # o
])


def _build_bass(t_steps=T):
    from contextlib import ExitStack
    from concourse import bacc, mybir, library_config
    import concourse.tile as tile

    f16, f32, i16 = mybir.dt.float16, mybir.dt.float32, mybir.dt.int16
    Sig = mybir.ActivationFunctionType.Sigmoid
    Tanh = mybir.ActivationFunctionType.Tanh
    mult, add = mybir.AluOpType.mult, mybir.AluOpType.add

    NW = t_steps // S
    NI = S * BL            # 256 gathered rows per window, t-major

    nc = bacc.Bacc("TRN2", target_bir_lowering=False, debug=False)
    idx_d = nc.declare_dram_parameter("idx", [128, NW, NI // 16], i16, isOutput=False)
    tab_d = nc.declare_dram_parameter("table", [V, H], f16, isOutput=False)
    wih_d = nc.declare_dram_parameter("wih_t", [H, G4], f16, isOutput=False)
    whh_d = nc.declare_dram_parameter("whh_t", [H, G4], f16, isOutput=False)
    b_d = nc.declare_dram_parameter("bias", [2, 4, 128], f16, isOutput=False)
    ind_d = nc.declare_dram_parameter("ind", [2, 2 * NI], f16, isOutput=False)
    h0_d = nc.declare_dram_parameter("h0t", [128, K, BL], f16, isOutput=False)
    c0_d = nc.declare_dram_parameter("c0t", [128, K, BL], f32, isOutput=False)
    ho_d = nc.declare_dram_parameter("h_out", [128, K, BL], f32, isOutput=True)
    co_d = nc.declare_dram_parameter("c_out", [128, K, BL], f32, isOutput=True)

    import bass_rust

    with tile.TileContext(nc) as tc, ExitStack() as ctx:
        const = ctx.enter_context(tc.tile_pool(name="const", bufs=1))
        embp = ctx.enter_context(tc.tile_pool(name="embp", bufs=3))
        psum = ctx.enter_context(tc.tile_pool(name="psum", bufs=2, space="PSUM"))
        sp = ctx.enter_context(tc.tile_pool(name="sp", bufs=3))
        tmp = ctx.enter_context(tc.tile_pool(name="tmp", bufs=3))
        hp = ctx.enter_context(tc.tile_pool(name="hp", bufs=3))

        # idx upload + library load first so the first gather's Q7 work
        # overlaps the remaining constant DMAs
        idx_sb = const.tile([128, NW, NI // 16], i16, name="idx_sb")
        nc.sync.dma_start(idx_sb[:], idx_d[:])
        nc.gpsimd.load_library(library_config.mlp)
        whh_sb, wih_sb = [], []
        for k in range(K):
            wt = const.tile([128, G4], f16, name=f"whh_sb{k}")
            nc.sync.dma_start(wt[:], whh_d[128 * k:128 * (k + 1), :])
            whh_sb.append(wt)
            xt = const.tile([128, G4], f16, name=f"wih_sb{k}")
            nc.sync.dma_start(xt[:], wih_d[128 * k:128 * (k + 1), :])
            wih_sb.append(xt)
        b_sb = const.tile([2, 4, 128], f16, name="b_sb")
        nc.sync.dma_start(b_sb[:], b_d[:])
        ind = const.tile([2, 2 * NI], f16, name="ind")
        nc.sync.dma_start(ind[:], ind_d[:])

        # state: c (fp32, K chunks), h (fp16)
        ct = const.tile([128, K, BL], f32, name="ct")
        nc.sync.dma_start(ct[:], c0_d[:])
        h_cur = const.tile([128, K, BL], f16, name="h0_sb")
        nc.sync.dma_start(h_cur[:], h0_d[:])

        embt = {}
        ps = {}

        def gather(w):
            e = embp.tile([128, K, NI], f16, name="embt", tag=f"embt{w % 3}",
                          bufs=1)
            g_i = nc.gpsimd.dma_gather(
                out_ap=e[:], in_ap=tab_d[:],
                idxs_ap=idx_sb[:, w, :],
                num_idxs=NI, num_idxs_reg=NI, elem_size=H, transpose=True)
            embt[w] = e
            return g_i

        # Tile tracks PSUM hazards per-tile, so each window's PSUM is split
        # into three tiles by gate group (f,i | g | o): the gate activations
        # then only wait on their own group's matmuls, and the next window's
        # writes only conflict with their own group's reads.
        def _mtile(w, m):
            pfi, pg, po = ps[w]
            if m < 2 * K:
                return pfi, m
            if m < 3 * K:
                return pg, m - 2 * K
            return po, m - 3 * K

        def alloc_ps(w):
            pfi = psum.tile([128, 2 * K, S, BL], f32, name="pfi",
                            tag=f"pfi{w % 2}", bufs=1)
            pg = psum.tile([128, K, S, BL], f32, name="pg",
                           tag=f"pg{w % 2}", bufs=1)
            po = psum.tile([128, K, S, BL], f32, name="po",
                           tag=f"po{w % 2}", bufs=1)
            ps[w] = (pfi, pg, po)

        def bias_mms(w, lo, hi, after=None):
            # first write to each bank: full-bank N=512 matmul, start=True
            if lo == 0 and w not in ps:
                alloc_ps(w)
            for b in range(lo, hi):
                tl, lm = _mtile(w, 2 * b)
                mm = nc.tensor.matmul(
                    out=tl[:, lm:lm + 2, :, :],
                    lhsT=b_sb[:, b, :],
                    rhs=ind[:], start=True, stop=False, skip_group_check=True)
                if after is not None:
                    bass_rust.add_dep_helper(mm.ins, after.ins, sync=False,
                                             reason="pin bias after burst")

        def x_mms(w, lo, hi, after=None):
            for j in range(lo, hi):
                m, k = j // K, j % K
                tl, lm = _mtile(w, m)
                mm = nc.tensor.matmul(
                    out=tl[:, lm, :, :],
                    lhsT=wih_sb[k][:, 128 * m:128 * (m + 1)],
                    rhs=embt[w][:, k, :],
                    start=False, stop=False, skip_group_check=True)
                if after is not None:
                    bass_rust.add_dep_helper(mm.ins, after.ins, sync=False,
                                             reason="pin x after burst")

        def burst(w, s, mlo, mhi):
            # m-major, g chunks first, so tanh(g) can start mid-burst
            last = None
            for m in range(mlo, mhi):
                for k in range(K):
                    tl, lm = _mtile(w, m)
                    last = nc.tensor.matmul(
                        out=tl[:, lm, s, :],
                        lhsT=whh_sb[k][:, 128 * m:128 * (m + 1)],
                        rhs=h_cur[:, k, :],
                        start=False, stop=(k == K - 1), skip_group_check=True)
            return last

        # prologue: window 0 fully prepared, window 1 gathered
        gather(0)
        if NW > 1:
            gather(1)
        # PE p-state warm-up: ~35 sustained N=512 matmuls trigger the clock
        # ramp 1.2 -> 2.4 GHz (measured: needs ~5us of continuous matmul;
        # hysteresis then holds the high clock through the per-step PE idle
        # gaps for the rest of the kernel). Overlaps the prologue DMAs and
        # the first gather; the results are garbage and are overwritten by
        # the start=True bias matmuls.
        alloc_ps(0)
        for _ in range(24):
            nc.tensor.matmul(
                out=ps[0][0][:, 0:2, :, :], lhsT=whh_sb[0][:, 0:128],
                rhs=whh_sb[0][:, 0:512], start=True, stop=True,
                skip_group_check=True)
        bias_mms(0, 0, 4)
        x_mms(0, 0, M * K)

        for w in range(NW):
            for s in range(S):
                t = w * S + s
                # ACT ops are emitted interleaved into the burst so their
                # PE-semaphore wait thresholds stay minimal (Tile computes
                # thresholds from program order).
                burst(w, s, 0, 2 * K)                # f, i chunks
                sall = sp.tile([128, 3 * K, BL], f16, name="sall", tag="sall")
                mmfc = tmp.tile([128, K, BL], f32, name="mmfc", tag="mmfc")
                mmit = tmp.tile([128, K, BL], f16, name="mmit", tag="mmit")
                tgt = tmp.tile([128, K, BL], f16, name="tgt", tag="tgt")
                tch = tmp.tile([128, K, BL], f16, name="tct", tag="tct")
                nc.scalar.activation(sall[:, 0:2 * K, :],
                                     ps[w][0][:, :, s, :], Sig)
                burst(w, s, 2 * K, 3 * K)            # g chunks
                nc.scalar.activation(tgt[:], ps[w][1][:, :, s, :], Tanh)
                last_mm = burst(w, s, 3 * K, 4 * K)  # o chunks
                nc.scalar.activation(sall[:, 2 * K:3 * K, :],
                                     ps[w][2][:, :, s, :], Sig)
                # DVE: f*c overlaps the tanh(g) ACT op; i*tg is all-fp16 (2x)
                nc.vector.tensor_tensor(
                    mmfc[:], sall[:, 0:K, :], ct[:, 0:K, :], mult)
                nc.vector.tensor_tensor(
                    mmit[:], sall[:, K:2 * K, :], tgt[:], mult)
                nc.vector.tensor_tensor(
                    ct[:, 0:K, :], mmfc[:], mmit[:], add)
                nc.scalar.activation(tch[:], ct[:, 0:K, :], Tanh)
                if t < t_steps - 1:
                    hn = hp.tile([128, K, BL], f16, name="hn", tag="hn")
                    nc.vector.tensor_tensor(
                        hn[:], sall[:, 2 * K:3 * K, :], tch[:], mult)
                    h_cur = hn
                else:
                    hf = tmp.tile([128, K, BL], f32, name="hf", tag="hf")
                    nc.vector.tensor_tensor(
                        hf[:], sall[:, 2 * K:3 * K, :], tch[:], mult)
                    nc.sync.dma_start(ho_d[:], hf[:])
                    nc.sync.dma_start(co_d[:], ct[:, 0:K, :])
                # window w+1 compute prep + window w+2 gather, spread across
                # this window's steps (done by s=6 so the s=7 -> s=0 handoff
                # is clean). X/bias matmuls are pinned behind this step's
                # recurrent burst so they fill the PE-idle tail.
                if w + 1 < NW:
                    if s == 0:
                        if w + 2 < NW:
                            gather(w + 2)
                        bias_mms(w + 1, 0, 2, after=last_mm)
                    elif s == 1:
                        bias_mms(w + 1, 2, 4, after=last_mm)
                    elif s <= 6:
                        n_x = M * K
                        lo = (s - 2) * n_x // 5
                        hi = (s - 1) * n_x // 5
                        x_mms(w + 1, lo, hi, after=last_mm)
                # PE keep-warm: garbage start=True matmuls into this window's
                # already-consumed PSUM strips (s' < s-1; re-initialized by
                # the next-next window's bias matmuls). Keeps the PE
                # utilization high enough that the 2.4 GHz p-state from the
                # prologue warm-up holds through the per-step idle gaps.
                # (start=False: a start=True would zero the whole 2KB bank,
                # wiping the still-live strips of the same bank.)
                if s >= 2:
                    for j in range(M):
                        tl, lm = _mtile(w, j)
                        dm = nc.tensor.matmul(
                            out=tl[:, lm, 0:s - 1, :],
                            lhsT=whh_sb[0][:, 0:128],
                            rhs=whh_sb[0][:, 0:(s - 1) * BL],
                            start=False, stop=False, skip_group_check=True)
                        bass_rust.add_dep_helper(
                            dm.ins, last_mm.ins, sync=False,
                            reason="pin keep-warm after burst")
            if w > 0:
                ps.pop(w - 1, None)
                embt.pop(w - 1, None)
    nc.finalize()
    return nc


def _prep_inputs(enc_inputs, h0, c0, embed, W_ih, W_hh, b_ih, b_hh, t_steps=T):
    """Host-side shard + layout prep. Returns list of per-core in_maps."""
    Wih_p = W_ih[_PERM].astype(np.float32)
    Whh_p = W_hh[_PERM].astype(np.float32)
    b_p = (b_ih + b_hh)[_PERM].astype(np.float32)
    wih_t = np.ascontiguousarray(Wih_p.T).astype(np.float16)   # [H, 4H]
    whh_t = np.ascontiguousarray(Whh_p.T).astype(np.float16)
    # bias packed per PSUM bank: bank b covers chunks 2b, 2b+1
    bias = np.ascontiguousarray(
        b_p.astype(np.float16).reshape(4, 2, 128).transpose(1, 0, 2))
    table = embed.astype(np.float16)                           # [V, H]
    NI = S * BL
    ind = np.zeros((2, 2 * NI), np.float16)
    for j in range(2):
        ind[j, NI * j:NI * (j + 1)] = 1.0

    NW = t_steps // S
    in_maps = []
    for c in range(N_CORES):
        wrapped = np.empty((128, NW, NI // 16), np.int16)
        for w in range(NW):
            bs = slice(c * BL, (c + 1) * BL)
            flat = enc_inputs[w * S:(w + 1) * S, bs].astype(np.int16).reshape(-1)
            w16 = flat.reshape(-1, 16).T                       # [16, 16]
            wrapped[:, w, :] = np.tile(w16, (8, 1))
        bs = slice(c * BL, (c + 1) * BL)
        h0t = np.empty((128, K, BL), np.float16)
        c0t = np.empty((128, K, BL), np.float32)
        for k in range(K):
            h0t[:, k, :] = h0[bs].T[128 * k:128 * (k + 1), :]
            c0t[:, k, :] = c0[bs].T[128 * k:128 * (k + 1), :]
        in_maps.append({
            "idx": np.ascontiguousarray(wrapped), "table": table,
            "wih_t": wih_t, "whh_t": whh_t,
            "bias": bias, "ind": ind, "h0t": h0t, "c0t": c0t,
        })
    return in_maps


def _unshard(results):
    h = np.empty((B, H), np.float32)
    c = np.empty((B, H), np.float32)
    for core, out in enumerate(results):
        bs = slice(core * BL, (core + 1) * BL)
        for k in range(K):
            h[bs, 128 * k:128 * (k + 1)] = out["h_out"][:, k, :].T
            c[bs, 128 * k:128 * (k + 1)] = out["c_out"][:, k, :].T
    return h, c


def kernel(enc_inputs, h0, c0, embed, W_ih, W_hh, b_ih, b_hh):
    from concourse.bass_utils import run_bass_kernel_spmd

    enc_inputs = np.asarray(enc_inputs)
    h0 = np.asarray(h0, dtype=np.float32)
    c0 = np.asarray(c0, dtype=np.float32)
    embed = np.asarray(embed, dtype=np.float32)
    W_ih = np.asarray(W_ih, dtype=np.float32)
    W_hh = np.asarray(W_hh, dtype=np.float32)
    b_ih = np.asarray(b_ih, dtype=np.float32)
    b_hh = np.asarray(b_hh, dtype=np.float32)

    nc = _build_bass()
    in_maps = _prep_inputs(enc_inputs, h0, c0, embed, W_ih, W_hh, b_ih, b_hh)
    res = run_bass_kernel_spmd(nc, in_maps, core_ids=list(range(N_CORES)))
    return _unshard(res.results)
